# revision 28
# baseline (speedup 1.0000x reference)
"""Distributed GQA attention block for Trainium2 (8 NeuronCores).

Problem: nn_Attention_65927747993826
  x:[2,2048,2048] f32, causal GQA attention, H=32 query heads, G=8 KV groups,
  head_size=64, with q/k/v/out projections and bias.

Sharding (8-way head parallel): core c owns query heads [4c, 4c+4) and KV
group c. Each core computes q/k/v projections for its heads from the full x,
causal flash-attention for its 4 heads, and a partial output projection
through its 256 rows of Wo. The host sums the 8 partial outputs and adds the
bias (a per-feature constant commutes with the partial-sum reduction).

Layouts on chip are feature-major ("transposed"): x^T [E, S] etc., so every
matmul contracts over the partition dim with zero on-chip transposes except
v (PE-transposed). Compute dtype bf16 (f32 accumulate in PSUM).

Head-PAIR packing: q is stored with head 2j at partitions 0-63 and head
2j+1 at 64-127 (k duplicated across both halves), so the two heads' score
matmuls are K=64 row-tiles at tile_position (0,0)/(64,0) that execute
CONCURRENTLY on the PE's 32x32 subarrays — halving score cost vs the old
zero-padded K=128 scheme. Softmax denominators ride the ones-column of
v_aug; normalization is a reciprocal_approx_fast + a K=2 PE "broadcast"
matmul against a selector matrix + one full-width DVE multiply per chunk.
"""

from contextlib import ExitStack

import numpy as np
import ml_dtypes

import concourse.bass as bass
import concourse.mybir as mybir
import concourse.tile as tile
from concourse import bacc
from concourse.bass import ts, ds
from concourse.bass_utils import run_bass_kernel_spmd

B, S, E = 2, 2048, 2048
H, G, D = 32, 8, 64
NCORES = 8
HPC = H // NCORES            # query heads per core: 4
FPC = HPC * D                # q features per core: 256
P = 128
KT = E // P                  # 16 contraction tiles over E
NT = S // 512                # 4 token 512-blocks per batch
SCALE = D ** -0.5
F32 = mybir.dt.float32
BF16 = mybir.dt.bfloat16
FA = mybir.ActivationFunctionType
ALU = mybir.AluOpType


def build_nc():
    nc = bacc.Bacc()
    # x_t tiled [B, NT, E, 512]: each (b, n) token-block is a dense 2MB
    # region so the strided per-partition DMA rows stay page-local
    x_t = nc.declare_dram_parameter("x_t", [B, NT, E, 512], BF16, isOutput=False)
    wq = nc.declare_dram_parameter("wq", [E, FPC], BF16, isOutput=False)
    wkv = nc.declare_dram_parameter("wkv", [E, P], BF16, isOutput=False)
    wo = nc.declare_dram_parameter("wo", [FPC, E], BF16, isOutput=False)
    # host-baked constants: identity (v transposes), upper-triangular causal
    # mask, and the selector matrix for the normalize broadcast matmuls
    cst = nc.declare_dram_parameter("cst", [P, 3 * P], BF16, isOutput=False)
    # out tiled [B, KT, 2, 128, 1024]: every output DMA is one contiguous
    # 256KB block (two token-blocks per store); the host reassembles
    out = nc.declare_dram_parameter("out", [B, KT, 2, P, 1024], BF16, isOutput=True)

    with ExitStack() as ctx:
        tc = ctx.enter_context(tile.TileContext(nc))
        consts = ctx.enter_context(tc.tile_pool(name="consts", bufs=1))
        wpool = ctx.enter_context(tc.tile_pool(name="w", bufs=1))
        xbp = ctx.enter_context(tc.tile_pool(name="xb", bufs=4))
        qkvp = ctx.enter_context(tc.tile_pool(name="qkv", bufs=2))
        ppool = ctx.enter_context(tc.tile_pool(name="probs", bufs=14))
        npool = ctx.enter_context(tc.tile_pool(name="norm", bufs=4))
        opool = ctx.enter_context(tc.tile_pool(name="outsb", bufs=4))
        pp_mm = ctx.enter_context(tc.tile_pool(name="pmm", bufs=2, space="PSUM"))
        pp_sp = ctx.enter_context(tc.tile_pool(name="psp", bufs=2, space="PSUM"))
        pp_acc = ctx.enter_context(tc.tile_pool(name="pacc", bufs=2, space="PSUM"))

        # ---- critical-path weight chunk first: the opening matmul only
        # needs wq k-tiles 0-1 ----
        wq_sb = wpool.tile([P, KT, FPC], BF16)
        nc.gpsimd.dma_start(
            wq_sb[:, 0:2, :], wq[0:256, :].rearrange("(ko p) m -> p ko m", p=P)
        )

        # ---- constants (host-baked, DMA'd on the gpsimd queue) ----
        cst_sb = consts.tile([P, 3 * P], BF16)
        nc.gpsimd.dma_start(cst_sb, cst[:, :])
        ident = cst_sb[:, 0:P]
        tri = cst_sb[:, P : 2 * P]  # tri[k, q] = 1 iff q >= k
        # selector for the softmax-normalize broadcast matmuls: for chunk
        # kk, bc[:, kk*512:] = selc[64kk:64kk+33].T @ rec[64kk:64kk+33] puts
        # rec row 64kk on partitions 0-63 and row 64kk+32 on 64-127 (den
        # rows sit at 32-aligned partitions; engine APs need 32-aligned
        # bases). selc's zero rows hit dq's memset-1.0 filler, never NaN.
        selc = cst_sb[:, 2 * P : 3 * P]

        # ---- remaining weights (gpsimd DMA queue, parallel to x on
        # sync/scalar/vector) ----
        nc.gpsimd.dma_start(
            wq_sb[:, 2:, :], wq[256:, :].rearrange("(ko p) m -> p ko m", p=P)
        )
        wkv_sb = wpool.tile([P, KT, P], BF16)
        nc.gpsimd.dma_start(wkv_sb, wkv.rearrange("(ko p) m -> p ko m", p=P))
        wo_sb = wpool.tile([P, 2, E], BF16)
        nc.gpsimd.dma_start(wo_sb, wo.rearrange("(ko p) m -> p ko m", p=P))

        state = {}
        xq = {}  # b -> prefetched half-tiles for the next token block

        def issue_x(b, n):
            """issue the two half-tile loads for block (b, n), one per
            hardware DMA queue."""
            tiles = []
            for g in range(2):
                xb = xbp.tile([P, KT // 2, 512], BF16)
                (nc.sync if g == 0 else nc.scalar).dma_start(
                    xb,
                    x_t[b, n, ds(1024 * g, 1024), :].rearrange(
                        "(ko p) s -> p ko s", p=P
                    ),
                )
                tiles.append(xb)
            return tiles

        def gen_proj_n(b, n):
            """projection of token-block n for batch b (PE-heavy)."""
            if n == 0:
                # q4: head 2j at partitions 0-63, head 2j+1 at 64-127 so the
                # two heads' score matmuls row-tile the PE concurrently
                q4 = qkvp.tile([P, 2, S], BF16, tag="q4")
                # k2: k^T duplicated on both partition halves (scores lhsT)
                k2 = qkvp.tile([P, S], BF16, tag="k2")
                # kv: k^T rows 0-63, v^T rows 64-127 (v-transpose source)
                kvsb = qkvp.tile([P, S], BF16, tag="kv")
                # v token-major (+ ones column), PE-transposed per block
                vsb = qkvp.tile([P, S // P, D + 1], BF16, tag="v")
                nc.gpsimd.memset(vsb[:, :, D : D + 1], 1.0)
                # attnsb holds UNNORMALIZED attn until deferred normalize
                attnsb = qkvp.tile([P, 2, S], BF16, tag="attn")
                state[b] = (q4, k2, kvsb, vsb, attnsb)
            q4, k2, kvsb, vsb, attnsb = state[b]
            # x half-tiles are prefetched one block ahead (issued before
            # the PREVIOUS block's m-loop) so transfers hide under compute.
            # The very first block is chunked tile-by-tile across both
            # hardware DMA queues so the opening matmuls track arrivals.
            if b == 0 and n == 0:
                xh = []
                xb = xbp.tile([P, KT // 2, 512], BF16)
                for q_ in range(8):
                    (nc.sync if q_ % 2 == 0 else nc.scalar).dma_start(
                        xb[:, ds(q_, 1), :],
                        x_t[b, n, ds(q_ * 128, 128), :].rearrange(
                            "(ko p) s -> p ko s", p=P
                        ),
                    )
                xh.append(xb)
                xb = xbp.tile([P, KT // 2, 512], BF16)
                for q_ in range(4):
                    (nc.sync if q_ % 2 == 0 else nc.scalar).dma_start(
                        xb[:, ds(q_ * 2, 2), :],
                        x_t[b, n, ds(1024 + q_ * 256, 256), :].rearrange(
                            "(ko p) s -> p ko s", p=P
                        ),
                    )
                xh.append(xb)
            else:
                xh = xq[b]
            if n < 3:
                xq[b] = issue_x(b, n + 1)
            elif b == 0:
                xq[1] = issue_x(1, 0)
            if b == 0 and n == 0:
                # k-major so the PE consumes each arriving x chunk three
                # times over (q pair 0, q pair 1, kv) before waiting for the
                # next chunk; the two extra accumulators borrow the (idle)
                # score-PSUM pool
                ps0 = pp_mm.tile([P, 512], F32, tag="mm")
                ps12 = pp_sp.tile([P, 1024], F32, tag="sp")
                psl = [ps0, ps12[:, ts(0, 512)], ps12[:, ts(1, 512)]]
                for k in range(KT):
                    for m in range(3):
                        lhsT = wq_sb[:, k, ts(m, P)] if m < 2 else wkv_sb[:, k, :]
                        nc.tensor.matmul(
                            psl[m],
                            lhsT,
                            xh[k // 8][:, k % 8, :],
                            start=(k == 0),
                            stop=(k == KT - 1),
                        )
                    if k % 8 == 5:
                        yield
                for m in range(3):
                    ps = psl[m]
                    if m < 2:
                        nc.vector.tensor_copy(q4[:, m, ts(n, 512)], ps)
                    else:
                        nc.vector.tensor_copy(kvsb[:, ts(n, 512)], ps)
                        nc.vector.tensor_copy(k2[0:64, ts(n, 512)], ps[0:64, :])
                        nc.vector.tensor_copy(k2[64:128, ts(n, 512)], ps[0:64, :])
                yield
            else:
                for m in range(3):
                    ps = pp_mm.tile([P, 512], F32, tag="mm")
                    for k in range(KT):
                        lhsT = wq_sb[:, k, ts(m, P)] if m < 2 else wkv_sb[:, k, :]
                        nc.tensor.matmul(
                            ps,
                            lhsT,
                            xh[k // 8][:, k % 8, :],
                            start=(k == 0),
                            stop=(k == KT - 1),
                        )
                    if m < 2:
                        # heads 2m (rows 0-63) and 2m+1 (rows 64-127) in place
                        nc.vector.tensor_copy(q4[:, m, ts(n, 512)], ps)
                    else:
                        nc.vector.tensor_copy(kvsb[:, ts(n, 512)], ps)
                        nc.vector.tensor_copy(k2[0:64, ts(n, 512)], ps[0:64, :])
                        nc.vector.tensor_copy(k2[64:128, ts(n, 512)], ps[0:64, :])
                    yield
            # v transposes for this token-block, one group per n so the
            # attention for q-block n can start as soon as proj n is done
            for st in range(4 * n, 4 * n + 4):
                tp = pp_mm.tile([P, P], BF16, tag="mm")
                nc.tensor.transpose(tp, kvsb[:, ts(st, P)], ident)
                nc.vector.tensor_copy(vsb[:, st, 0:D], tp[:, 64:128])
            yield

        def gen_att_qt(b, qt):
            """causal attention for q-block qt of batch b (ACT-heavy: exp).

            Heads run in pairs: head 2j streams from partitions 0-63, head
            2j+1 from 64-127, two concurrent K=64 PE row-tiles per k-tile.
            softmax denominators come for free from the ones-column of v_aug
            (row 64 of each accumulator); normalization is deferred to a
            batched reciprocal + PE-broadcast per q-block.
            """
            q4, k2, kvsb, vsb, attnsb = state[b]
            dq = npool.tile([P, 512], F32, tag="den")
            nc.gpsimd.memset(dq, 1.0)
            nfull = 4 * qt
            for j in range(2):
                accA = pp_acc.tile([D + 1, 512], F32, tag="acc")
                accB = pp_acc.tile([D + 1, 512], F32, tag="acc")
                # 1) diagonal scores first: their exp+mask chains get
                #    maximum slack before their attnV consumers issue last.
                #    Head A lands at sp cols [0,w), head B at [512,512+w);
                #    one exp ACTIVATE spans both (the gap cols are garbage
                #    that nothing reads).
                diag_prs = []
                for t in range(4):
                    kt = nfull + t
                    off = t * P
                    w_ = 512 - off
                    sp = pp_sp.tile([P, 1024], F32, tag="sp")
                    nc.tensor.matmul(
                        sp[:, ds(0, w_)],
                        k2[0:64, ts(kt, P)],
                        q4[0:64, j, ds(512 * qt + off, w_)],
                        start=True,
                        stop=True,
                    )
                    nc.tensor.matmul(
                        sp[:, ds(512, w_)],
                        k2[64:128, ts(kt, P)],
                        q4[64:128, j, ds(512 * qt + off, w_)],
                        start=True,
                        stop=True,
                    )
                    pr = ppool.tile([P, 1024], BF16, tag="pr")
                    diag_prs.append(pr)
                    nc.scalar.activation(
                        pr[:, ds(0, 512 + w_)],
                        sp[:, ds(0, 512 + w_)],
                        FA.Exp,
                        scale=SCALE,
                    )
                    nc.vector.tensor_tensor(
                        pr[:, ds(0, P)], pr[:, ds(0, P)], tri, ALU.mult
                    )
                    nc.vector.tensor_tensor(
                        pr[:, ds(512, P)], pr[:, ds(512, P)], tri, ALU.mult
                    )
                yield
                # 2) full (off-diagonal) k-tiles: one [128,1024] PSUM tile
                #    holds both heads' scores for k-tile kt; one exp covers
                #    both; attnV for each head follows immediately
                firstA = firstB = True
                for kt in range(nfull):
                    sp = pp_sp.tile([P, 1024], F32, tag="sp")
                    nc.tensor.matmul(
                        sp[:, ts(0, 512)],
                        k2[0:64, ts(kt, P)],
                        q4[0:64, j, ts(qt, 512)],
                        start=True,
                        stop=True,
                    )
                    nc.tensor.matmul(
                        sp[:, ts(1, 512)],
                        k2[64:128, ts(kt, P)],
                        q4[64:128, j, ts(qt, 512)],
                        start=True,
                        stop=True,
                    )
                    pr = ppool.tile([P, 1024], BF16, tag="pr")
                    nc.scalar.activation(pr, sp, FA.Exp, scale=SCALE)
                    nc.tensor.matmul(
                        accA, vsb[:, kt, :], pr[:, ts(0, 512)],
                        start=firstA, stop=False,
                    )
                    firstA = False
                    nc.tensor.matmul(
                        accB, vsb[:, kt, :], pr[:, ts(1, 512)],
                        start=firstB, stop=False,
                    )
                    firstB = False
                    if kt % 2 == 1:
                        yield
                # 3) diagonal attnV last
                for t in range(4):
                    kt = nfull + t
                    off = t * P
                    w_ = 512 - off
                    nc.tensor.matmul(
                        accA[:, ds(off, w_)],
                        vsb[:, kt, :],
                        diag_prs[t][:, ds(0, w_)],
                        start=firstA,
                        stop=(t == 3),
                    )
                    firstA = False
                    nc.tensor.matmul(
                        accB[:, ds(off, w_)],
                        vsb[:, kt, :],
                        diag_prs[t][:, ds(512, w_)],
                        start=firstB,
                        stop=(t == 3),
                    )
                    firstB = False
                yield
                # evacuate unnormalized attn + denominator rows (chunk j's
                # denominators land at 32-aligned partitions 64j/64j+32 for
                # the selector-matmul broadcast below)
                nc.vector.tensor_copy(attnsb[0:64, j, ts(qt, 512)], accA[0:D, :])
                nc.vector.tensor_copy(dq[64 * j : 64 * j + 1, :], accA[D : D + 1, :])
                nc.vector.tensor_copy(
                    attnsb[64:128, j, ts(qt, 512)], accB[0:D, :]
                )
                nc.vector.tensor_copy(
                    dq[64 * j + 32 : 64 * j + 33, :], accB[D : D + 1, :]
                )
                yield
            # deferred softmax normalization for this q-block: fast approx
            # reciprocal (denominators are >= e^-8, ~51 ULP is plenty), then
            # two concurrent K=2 broadcast matmuls spread the four rec rows
            # across the partitions of both chunks, then two full-width mults
            rec = npool.tile([P, 512], F32, tag="rec")
            nc.vector.reciprocal_approx_fast(out=rec, in_=dq)
            # bf16 copy for the broadcast matmuls (fp32 PE matmul is 2-pass)
            recb = npool.tile([P, 512], BF16, tag="recb")
            nc.vector.tensor_copy(recb, rec)
            bc = pp_sp.tile([P, 1024], F32, tag="sp")
            nc.tensor.matmul(
                bc[:, ts(0, 512)], selc[0:33, :], recb[0:33, :],
                start=True, stop=True,
            )
            nc.tensor.matmul(
                bc[:, ts(1, 512)], selc[64:97, :], recb[64:97, :],
                start=True, stop=True,
            )
            for kk in range(2):
                dst = attnsb[:, kk, ts(qt, 512)]
                nc.vector.tensor_tensor(dst, dst, bc[:, ts(kk, 512)], ALU.mult)
            yield

        def gen_outproj(b, use_act):
            """partial output projection for batch b (PE-heavy).

            npair-outer so the later token blocks' softmax-normalize latency
            is hidden behind the first pair's matmuls. use_act alternates the
            PSUM evacuation onto ScalarE only when no attention phase is
            keeping ScalarE saturated with exps. Each store covers two token
            blocks (256KB) to halve the sync queue's DMA-issue load.
            """
            attnsb = state[b][4]
            for np_ in range(2):
                for m in range(KT):
                    osb = opool.tile([P, 1024], BF16)
                    for nn in range(2):
                        po = pp_mm.tile([P, 512], F32, tag="mm")
                        for kk in range(2):
                            nc.tensor.matmul(
                                po,
                                wo_sb[:, kk, ts(m, P)],
                                attnsb[:, kk, ts(2 * np_ + nn, 512)],
                                start=(kk == 0),
                                stop=(kk == 1),
                            )
                        if (m + nn) % 3 == 2 or (use_act and (m + nn) % 3 == 1):
                            nc.scalar.copy(osb[:, ts(nn, 512)], po)
                        else:
                            nc.vector.tensor_copy(osb[:, ts(nn, 512)], po)
                    nc.sync.dma_start(out[b, m, np_, :, :], osb)
                    if m % 2 == 1:
                        yield

        def run_all(gen):
            for _ in gen:
                pass

        def interleave(pairs, stop_idx=None):
            """pairs: list of [gen, steps_per_round]. Round-robin with ratios
            so the PE-filler generator is spread across the whole phase.
            With stop_idx set, the phase ends when that generator exhausts
            (the others keep their state for the next phase)."""
            live = [[g, r] for g, r in pairs]
            stop = live[stop_idx][0] if stop_idx is not None else None
            while live:
                for gr in live[:]:
                    try:
                        for _ in range(gr[1]):
                            next(gr[0])
                    except StopIteration:
                        live.remove(gr)
                        if gr[0] is stop:
                            return

        def delayed(gen, k):
            for _ in range(k):
                yield
            yield from gen

        def chain(gens):
            for g in gens:
                yield from g

        # Pipeline projections, attention, and out-projections so PE-heavy
        # matmul work fills the PE bubbles of the ACT(exp)-bound attention
        # phases. Attention for a batch trails its projection by ~1 token
        # block; out-projections enter delayed so their matmuls trail the
        # q-block normalizes they depend on in the in-order PE stream
        # (too-small delays DEADLOCK: PE waits on a DVE normalize whose
        # feeding matmul sits behind it in the queue).
        # NB: abandoned delayed() wrappers must keep being stepped by name —
        # letting one get garbage-collected closes the delegated generator.
        p0 = chain([gen_proj_n(0, n) for n in range(NT)])
        p1 = chain([gen_proj_n(1, n) for n in range(NT)])
        a0 = delayed(chain([gen_att_qt(0, qt) for qt in range(NT)]), 6)
        a1 = chain([gen_att_qt(1, qt) for qt in range(NT)])
        op0 = delayed(gen_outproj(0, False), 8)
        op1 = delayed(gen_outproj(1, True), 12)
        a1d = delayed(a1, 10)
        for _ in range(4):
            next(p0)  # proj(0) n=0
        interleave([(p0, 1), (a0, 2)], stop_idx=0)
        interleave([(a0, 3), (p1, 1), (op0, 1), (a1d, 2)], stop_idx=0)
        interleave([(a1d, 3), (p1, 1), (op0, 1), (op1, 1)], stop_idx=0)
        run_all(op1)
    return nc


BF = ml_dtypes.bfloat16


def make_in_maps(x, Wq, Wk, Wv, Wo):
    # [B, S, E] -> [B, NT, E, 512] (token-block-tiled, feature-major)
    x_t = np.ascontiguousarray(
        np.transpose(
            np.asarray(x, np.float32).reshape(B, NT, 512, E), (0, 1, 3, 2)
        )
    ).astype(BF)
    Wq = np.asarray(Wq, np.float32)
    Wk = np.asarray(Wk, np.float32)
    Wv = np.asarray(Wv, np.float32)
    Wo = np.asarray(Wo, np.float32)
    # constants: [ident | tri | selc] bf16
    cst = np.zeros((P, 3 * P), np.float32)
    cst[:, 0:P] = np.eye(P)
    cst[:, P : 2 * P] = np.triu(np.ones((P, P)))  # tri[k, q] = 1 iff q >= k
    for r0 in (0, 64):
        cst[r0, 2 * P : 2 * P + 64] = 1.0
        cst[r0 + 32, 2 * P + 64 : 3 * P] = 1.0
    cst = cst.astype(BF)
    in_maps = []
    for c in range(NCORES):
        wq_sh = np.ascontiguousarray(Wq[:, FPC * c : FPC * (c + 1)]).astype(BF)
        wkv_sh = np.concatenate(
            [Wk[:, D * c : D * (c + 1)], Wv[:, D * c : D * (c + 1)]], axis=1
        ).astype(BF)
        wo_sh = np.ascontiguousarray(Wo[FPC * c : FPC * (c + 1), :]).astype(BF)
        in_maps.append(
            {"x_t": x_t, "wq": wq_sh, "wkv": wkv_sh, "wo": wo_sh, "cst": cst}
        )
    return in_maps


_NC_CACHE = {}


def get_nc():
    if "nc" not in _NC_CACHE:
        nc = build_nc()
        nc.compile()
        _NC_CACHE["nc"] = nc
    return _NC_CACHE["nc"]


def kernel(x, Wq, Wk, Wv, Wo, bo, mask=None, **_ignored):
    nc = get_nc()
    in_maps = make_in_maps(x, Wq, Wk, Wv, Wo)
    res = run_bass_kernel_spmd(nc, in_maps, list(range(NCORES)))
    total = np.zeros((B, KT, 2, P, 1024), np.float32)
    for c in range(NCORES):
        total += np.asarray(res.results[c]["out"], np.float32)
    # [B, KT, 2, 128, 1024] -> [B, S, E]: feature = m*128+p, token = np*1024+s
    full = np.transpose(total, (0, 2, 4, 1, 3)).reshape(B, S, E)
    full = full + np.asarray(bo, np.float32)[None, None, :]
    return np.ascontiguousarray(full)


# revision 31
# speedup vs baseline: 1.0091x; 1.0091x over previous
"""Distributed GQA attention block for Trainium2 (8 NeuronCores).

Problem: nn_Attention_65927747993826
  x:[2,2048,2048] f32, causal GQA attention, H=32 query heads, G=8 KV groups,
  head_size=64, with q/k/v/out projections and bias.

Sharding (8-way head parallel): core c owns query heads [4c, 4c+4) and KV
group c. Each core computes q/k/v projections for its heads from the full x,
causal flash-attention for its 4 heads, and a partial output projection
through its 256 rows of Wo. The host sums the 8 partial outputs and adds the
bias (a per-feature constant commutes with the partial-sum reduction).

Layouts on chip are feature-major ("transposed"): x^T [E, S] etc., so every
matmul contracts over the partition dim with zero on-chip transposes except
v (PE-transposed). Compute dtype bf16 (f32 accumulate in PSUM).

Head-PAIR packing: q is stored with head 2j at partitions 0-63 and head
2j+1 at 64-127 (k duplicated across both halves), so the two heads' score
matmuls are K=64 row-tiles at tile_position (0,0)/(64,0) that execute
CONCURRENTLY on the PE's 32x32 subarrays — halving score cost vs the old
zero-padded K=128 scheme. Softmax denominators ride the ones-column of
v_aug; normalization is a reciprocal_approx_fast + a K=2 PE "broadcast"
matmul against a selector matrix + one full-width DVE multiply per chunk.
"""

from contextlib import ExitStack

import numpy as np
import ml_dtypes

import concourse.bass as bass
import concourse.mybir as mybir
import concourse.tile as tile
from concourse import bacc
from concourse.bass import ts, ds
from concourse.bass_utils import run_bass_kernel_spmd

B, S, E = 2, 2048, 2048
H, G, D = 32, 8, 64
NCORES = 8
HPC = H // NCORES            # query heads per core: 4
FPC = HPC * D                # q features per core: 256
P = 128
KT = E // P                  # 16 contraction tiles over E
NT = S // 512                # 4 token 512-blocks per batch
SCALE = D ** -0.5
F32 = mybir.dt.float32
BF16 = mybir.dt.bfloat16
FA = mybir.ActivationFunctionType
ALU = mybir.AluOpType


def build_nc():
    nc = bacc.Bacc()
    # x_t tiled [B, NT, E, 512]: each (b, n) token-block is a dense 2MB
    # region so the strided per-partition DMA rows stay page-local
    x_t = nc.declare_dram_parameter("x_t", [B, NT, E, 512], BF16, isOutput=False)
    wq = nc.declare_dram_parameter("wq", [E, FPC], BF16, isOutput=False)
    wkv = nc.declare_dram_parameter("wkv", [E, P], BF16, isOutput=False)
    wo = nc.declare_dram_parameter("wo", [FPC, E], BF16, isOutput=False)
    # host-baked constants: identity (v transposes), upper-triangular causal
    # mask, and the selector matrix for the normalize broadcast matmuls
    cst = nc.declare_dram_parameter("cst", [P, 3 * P], BF16, isOutput=False)
    # out tiled [B, KT, 2, 128, 1024]: every output DMA is one contiguous
    # 256KB block (two token-blocks per store); the host reassembles
    out = nc.declare_dram_parameter("out", [B, KT, 2, P, 1024], BF16, isOutput=True)

    with ExitStack() as ctx:
        tc = ctx.enter_context(tile.TileContext(nc))
        consts = ctx.enter_context(tc.tile_pool(name="consts", bufs=1))
        wpool = ctx.enter_context(tc.tile_pool(name="w", bufs=1))
        xbp = ctx.enter_context(tc.tile_pool(name="xb", bufs=4))
        qkvp = ctx.enter_context(tc.tile_pool(name="qkv", bufs=2))
        ppool = ctx.enter_context(tc.tile_pool(name="probs", bufs=14))
        npool = ctx.enter_context(tc.tile_pool(name="norm", bufs=4))
        opool = ctx.enter_context(tc.tile_pool(name="outsb", bufs=4))
        pp_mm = ctx.enter_context(tc.tile_pool(name="pmm", bufs=2, space="PSUM"))
        pp_sp = ctx.enter_context(tc.tile_pool(name="psp", bufs=2, space="PSUM"))
        pp_acc = ctx.enter_context(tc.tile_pool(name="pacc", bufs=2, space="PSUM"))

        # ---- critical-path weight chunk first: the opening matmul only
        # needs wq k-tiles 0-1 ----
        wq_sb = wpool.tile([P, KT, FPC], BF16)
        nc.gpsimd.dma_start(
            wq_sb[:, 0:2, :], wq[0:256, :].rearrange("(ko p) m -> p ko m", p=P)
        )

        # ---- constants (host-baked, DMA'd on the gpsimd queue) ----
        cst_sb = consts.tile([P, 3 * P], BF16)
        nc.gpsimd.dma_start(cst_sb, cst[:, :])
        ident = cst_sb[:, 0:P]
        tri = cst_sb[:, P : 2 * P]  # tri[k, q] = 1 iff q >= k
        # selector for the softmax-normalize broadcast matmuls: for chunk
        # kk, bc[:, kk*512:] = selc[64kk:64kk+33].T @ rec[64kk:64kk+33] puts
        # rec row 64kk on partitions 0-63 and row 64kk+32 on 64-127 (den
        # rows sit at 32-aligned partitions; engine APs need 32-aligned
        # bases). selc's zero rows hit dq's memset-1.0 filler, never NaN.
        selc = cst_sb[:, 2 * P : 3 * P]

        # ---- remaining weights (gpsimd DMA queue, parallel to x on
        # sync/scalar/vector) ----
        nc.gpsimd.dma_start(
            wq_sb[:, 2:, :], wq[256:, :].rearrange("(ko p) m -> p ko m", p=P)
        )
        wkv_sb = wpool.tile([P, KT, P], BF16)
        nc.gpsimd.dma_start(wkv_sb, wkv.rearrange("(ko p) m -> p ko m", p=P))
        # wo on the scalar queue: its issue slot is free before the exps
        # start, and the data isn't needed until the first out-projection
        wo_sb = wpool.tile([P, 2, E], BF16)
        nc.scalar.dma_start(wo_sb, wo.rearrange("(ko p) m -> p ko m", p=P))

        state = {}
        xq = {}  # b -> prefetched half-tiles for the next token block

        def issue_x(b, n):
            """issue the two half-tile loads for block (b, n). g0 rides the
            gpsimd (software) queue — slower transfers, but issued a full
            block ahead and its issue cost lands on an idle engine; g1 on
            the sync hardware queue."""
            tiles = []
            for g in range(2):
                xb = xbp.tile([P, KT // 2, 512], BF16)
                (nc.gpsimd if g == 0 else nc.sync).dma_start(
                    xb,
                    x_t[b, n, ds(1024 * g, 1024), :].rearrange(
                        "(ko p) s -> p ko s", p=P
                    ),
                )
                tiles.append(xb)
            return tiles

        def gen_proj_n(b, n):
            """projection of token-block n for batch b (PE-heavy)."""
            if n == 0:
                # q4: head 2j at partitions 0-63, head 2j+1 at 64-127 so the
                # two heads' score matmuls row-tile the PE concurrently
                q4 = qkvp.tile([P, 2, S], BF16, tag="q4")
                # k2: k^T duplicated on both partition halves (scores lhsT)
                k2 = qkvp.tile([P, S], BF16, tag="k2")
                # kv: k^T rows 0-63, v^T rows 64-127 (v-transpose source)
                kvsb = qkvp.tile([P, S], BF16, tag="kv")
                # v token-major (+ ones column), PE-transposed per block
                vsb = qkvp.tile([P, S // P, D + 1], BF16, tag="v")
                nc.gpsimd.memset(vsb[:, :, D : D + 1], 1.0)
                # attnsb holds UNNORMALIZED attn until deferred normalize
                attnsb = qkvp.tile([P, 2, S], BF16, tag="attn")
                state[b] = (q4, k2, kvsb, vsb, attnsb)
            q4, k2, kvsb, vsb, attnsb = state[b]
            # x half-tiles are prefetched one block ahead (issued before
            # the PREVIOUS block's m-loop) so transfers hide under compute.
            # The very first block is chunked tile-by-tile across both
            # hardware DMA queues so the opening matmuls track arrivals.
            if b == 0 and n == 0:
                xh = []
                xb = xbp.tile([P, KT // 2, 512], BF16)
                for q_ in range(8):
                    (nc.sync if q_ % 2 == 0 else nc.scalar).dma_start(
                        xb[:, ds(q_, 1), :],
                        x_t[b, n, ds(q_ * 128, 128), :].rearrange(
                            "(ko p) s -> p ko s", p=P
                        ),
                    )
                xh.append(xb)
                xb = xbp.tile([P, KT // 2, 512], BF16)
                for q_ in range(4):
                    (nc.sync if q_ % 2 == 0 else nc.scalar).dma_start(
                        xb[:, ds(q_ * 2, 2), :],
                        x_t[b, n, ds(1024 + q_ * 256, 256), :].rearrange(
                            "(ko p) s -> p ko s", p=P
                        ),
                    )
                xh.append(xb)
            else:
                xh = xq[b]
            if n < 3:
                xq[b] = issue_x(b, n + 1)
            elif b == 0:
                xq[1] = issue_x(1, 0)
            if b == 0 and n == 0:
                # k-major so the PE consumes each arriving x chunk three
                # times over (q pair 0, q pair 1, kv) before waiting for the
                # next chunk; the two extra accumulators borrow the (idle)
                # score-PSUM pool
                ps0 = pp_mm.tile([P, 512], F32, tag="mm")
                ps12 = pp_sp.tile([P, 1024], F32, tag="sp")
                psl = [ps0, ps12[:, ts(0, 512)], ps12[:, ts(1, 512)]]
                for k in range(KT):
                    for m in range(3):
                        lhsT = wq_sb[:, k, ts(m, P)] if m < 2 else wkv_sb[:, k, :]
                        nc.tensor.matmul(
                            psl[m],
                            lhsT,
                            xh[k // 8][:, k % 8, :],
                            start=(k == 0),
                            stop=(k == KT - 1),
                        )
                    if k % 8 == 5:
                        yield
                for m in range(3):
                    ps = psl[m]
                    if m < 2:
                        nc.vector.tensor_copy(q4[:, m, ts(n, 512)], ps)
                    else:
                        nc.vector.tensor_copy(kvsb[:, ts(n, 512)], ps)
                        nc.vector.tensor_copy(k2[0:64, ts(n, 512)], ps[0:64, :])
                        nc.vector.tensor_copy(k2[64:128, ts(n, 512)], ps[0:64, :])
                yield
            else:
                for m in range(3):
                    ps = pp_mm.tile([P, 512], F32, tag="mm")
                    for k in range(KT):
                        lhsT = wq_sb[:, k, ts(m, P)] if m < 2 else wkv_sb[:, k, :]
                        nc.tensor.matmul(
                            ps,
                            lhsT,
                            xh[k // 8][:, k % 8, :],
                            start=(k == 0),
                            stop=(k == KT - 1),
                        )
                    if m < 2:
                        # heads 2m (rows 0-63) and 2m+1 (rows 64-127) in place
                        nc.vector.tensor_copy(q4[:, m, ts(n, 512)], ps)
                    else:
                        nc.vector.tensor_copy(kvsb[:, ts(n, 512)], ps)
                        nc.vector.tensor_copy(k2[0:64, ts(n, 512)], ps[0:64, :])
                        nc.vector.tensor_copy(k2[64:128, ts(n, 512)], ps[0:64, :])
                    yield
            # v transposes for this token-block, one group per n so the
            # attention for q-block n can start as soon as proj n is done
            for st in range(4 * n, 4 * n + 4):
                tp = pp_mm.tile([P, P], BF16, tag="mm")
                nc.tensor.transpose(tp, kvsb[:, ts(st, P)], ident)
                nc.vector.tensor_copy(vsb[:, st, 0:D], tp[:, 64:128])
            yield

        def gen_att_qt(b, qt):
            """causal attention for q-block qt of batch b (ACT-heavy: exp).

            Heads run in pairs: head 2j streams from partitions 0-63, head
            2j+1 from 64-127, two concurrent K=64 PE row-tiles per k-tile.
            softmax denominators come for free from the ones-column of v_aug
            (row 64 of each accumulator); normalization is deferred to a
            batched reciprocal + PE-broadcast per q-block.
            """
            q4, k2, kvsb, vsb, attnsb = state[b]
            dq = npool.tile([P, 512], F32, tag="den")
            nc.gpsimd.memset(dq, 1.0)
            nfull = 4 * qt
            for j in range(2):
                accA = pp_acc.tile([D + 1, 512], F32, tag="acc")
                accB = pp_acc.tile([D + 1, 512], F32, tag="acc")
                # 1) diagonal scores first: their exp+mask chains get
                #    maximum slack before their attnV consumers issue last.
                #    Head A lands at sp cols [0,w), head B at [512,512+w);
                #    one exp ACTIVATE spans both (the gap cols are garbage
                #    that nothing reads).
                diag_prs = []
                for t in range(4):
                    kt = nfull + t
                    off = t * P
                    w_ = 512 - off
                    sp = pp_sp.tile([P, 1024], F32, tag="sp")
                    nc.tensor.matmul(
                        sp[:, ds(0, w_)],
                        k2[0:64, ts(kt, P)],
                        q4[0:64, j, ds(512 * qt + off, w_)],
                        start=True,
                        stop=True,
                    )
                    nc.tensor.matmul(
                        sp[:, ds(512, w_)],
                        k2[64:128, ts(kt, P)],
                        q4[64:128, j, ds(512 * qt + off, w_)],
                        start=True,
                        stop=True,
                    )
                    pr = ppool.tile([P, 1024], BF16, tag="pr")
                    diag_prs.append(pr)
                    nc.scalar.activation(
                        pr[:, ds(0, 512 + w_)],
                        sp[:, ds(0, 512 + w_)],
                        FA.Exp,
                        scale=SCALE,
                    )
                    nc.vector.tensor_tensor(
                        pr[:, ds(0, P)], pr[:, ds(0, P)], tri, ALU.mult
                    )
                    nc.vector.tensor_tensor(
                        pr[:, ds(512, P)], pr[:, ds(512, P)], tri, ALU.mult
                    )
                yield
                # 2) full (off-diagonal) k-tiles: one [128,1024] PSUM tile
                #    holds both heads' scores for k-tile kt; one exp covers
                #    both; attnV for each head follows immediately
                firstA = firstB = True
                for kt in range(nfull):
                    sp = pp_sp.tile([P, 1024], F32, tag="sp")
                    nc.tensor.matmul(
                        sp[:, ts(0, 512)],
                        k2[0:64, ts(kt, P)],
                        q4[0:64, j, ts(qt, 512)],
                        start=True,
                        stop=True,
                    )
                    nc.tensor.matmul(
                        sp[:, ts(1, 512)],
                        k2[64:128, ts(kt, P)],
                        q4[64:128, j, ts(qt, 512)],
                        start=True,
                        stop=True,
                    )
                    pr = ppool.tile([P, 1024], BF16, tag="pr")
                    nc.scalar.activation(pr, sp, FA.Exp, scale=SCALE)
                    nc.tensor.matmul(
                        accA, vsb[:, kt, :], pr[:, ts(0, 512)],
                        start=firstA, stop=False,
                    )
                    firstA = False
                    nc.tensor.matmul(
                        accB, vsb[:, kt, :], pr[:, ts(1, 512)],
                        start=firstB, stop=False,
                    )
                    firstB = False
                    if kt % 2 == 1:
                        yield
                # 3) diagonal attnV last
                for t in range(4):
                    kt = nfull + t
                    off = t * P
                    w_ = 512 - off
                    nc.tensor.matmul(
                        accA[:, ds(off, w_)],
                        vsb[:, kt, :],
                        diag_prs[t][:, ds(0, w_)],
                        start=firstA,
                        stop=(t == 3),
                    )
                    firstA = False
                    nc.tensor.matmul(
                        accB[:, ds(off, w_)],
                        vsb[:, kt, :],
                        diag_prs[t][:, ds(512, w_)],
                        start=firstB,
                        stop=(t == 3),
                    )
                    firstB = False
                yield
                # evacuate unnormalized attn + denominator rows (chunk j's
                # denominators land at 32-aligned partitions 64j/64j+32 for
                # the selector-matmul broadcast below)
                nc.vector.tensor_copy(attnsb[0:64, j, ts(qt, 512)], accA[0:D, :])
                nc.vector.tensor_copy(dq[64 * j : 64 * j + 1, :], accA[D : D + 1, :])
                nc.vector.tensor_copy(
                    attnsb[64:128, j, ts(qt, 512)], accB[0:D, :]
                )
                nc.vector.tensor_copy(
                    dq[64 * j + 32 : 64 * j + 33, :], accB[D : D + 1, :]
                )
                yield
            # deferred softmax normalization for this q-block: fast approx
            # reciprocal (denominators are >= e^-8, ~51 ULP is plenty), then
            # two concurrent K=2 broadcast matmuls spread the four rec rows
            # across the partitions of both chunks, then two full-width mults
            rec = npool.tile([P, 512], F32, tag="rec")
            nc.vector.reciprocal_approx_fast(out=rec, in_=dq)
            # bf16 copy for the broadcast matmuls (fp32 PE matmul is 2-pass)
            recb = npool.tile([P, 512], BF16, tag="recb")
            nc.vector.tensor_copy(recb, rec)
            bc = pp_sp.tile([P, 1024], F32, tag="sp")
            nc.tensor.matmul(
                bc[:, ts(0, 512)], selc[0:33, :], recb[0:33, :],
                start=True, stop=True,
            )
            nc.tensor.matmul(
                bc[:, ts(1, 512)], selc[64:97, :], recb[64:97, :],
                start=True, stop=True,
            )
            for kk in range(2):
                dst = attnsb[:, kk, ts(qt, 512)]
                nc.vector.tensor_tensor(dst, dst, bc[:, ts(kk, 512)], ALU.mult)
            yield

        def gen_outproj(b, use_act):
            """partial output projection for batch b (PE-heavy).

            npair-outer so the later token blocks' softmax-normalize latency
            is hidden behind the first pair's matmuls. use_act alternates the
            PSUM evacuation onto ScalarE only when no attention phase is
            keeping ScalarE saturated with exps. Each store covers two token
            blocks (256KB) to halve the sync queue's DMA-issue load.
            """
            attnsb = state[b][4]
            for np_ in range(2):
                for m in range(KT):
                    osb = opool.tile([P, 1024], BF16)
                    for nn in range(2):
                        po = pp_mm.tile([P, 512], F32, tag="mm")
                        for kk in range(2):
                            nc.tensor.matmul(
                                po,
                                wo_sb[:, kk, ts(m, P)],
                                attnsb[:, kk, ts(2 * np_ + nn, 512)],
                                start=(kk == 0),
                                stop=(kk == 1),
                            )
                        if (m + nn) % 3 == 2 or (use_act and (m + nn) % 3 == 1):
                            nc.scalar.copy(osb[:, ts(nn, 512)], po)
                        else:
                            nc.vector.tensor_copy(osb[:, ts(nn, 512)], po)
                    nc.sync.dma_start(out[b, m, np_, :, :], osb)
                    if m % 2 == 1:
                        yield

        def run_all(gen):
            for _ in gen:
                pass

        def interleave(pairs, stop_idx=None):
            """pairs: list of [gen, steps_per_round]. Round-robin with ratios
            so the PE-filler generator is spread across the whole phase.
            With stop_idx set, the phase ends when that generator exhausts
            (the others keep their state for the next phase)."""
            live = [[g, r] for g, r in pairs]
            stop = live[stop_idx][0] if stop_idx is not None else None
            while live:
                for gr in live[:]:
                    try:
                        for _ in range(gr[1]):
                            next(gr[0])
                    except StopIteration:
                        live.remove(gr)
                        if gr[0] is stop:
                            return

        def delayed(gen, k):
            for _ in range(k):
                yield
            yield from gen

        def chain(gens):
            for g in gens:
                yield from g

        # Pipeline projections, attention, and out-projections so PE-heavy
        # matmul work fills the PE bubbles of the ACT(exp)-bound attention
        # phases. Attention for a batch trails its projection by ~1 token
        # block; out-projections enter delayed so their matmuls trail the
        # q-block normalizes they depend on in the in-order PE stream
        # (too-small delays DEADLOCK: PE waits on a DVE normalize whose
        # feeding matmul sits behind it in the queue).
        # NB: abandoned delayed() wrappers must keep being stepped by name —
        # letting one get garbage-collected closes the delegated generator.
        p0 = chain([gen_proj_n(0, n) for n in range(NT)])
        p1 = chain([gen_proj_n(1, n) for n in range(NT)])
        a0 = delayed(chain([gen_att_qt(0, qt) for qt in range(NT)]), 6)
        a1 = chain([gen_att_qt(1, qt) for qt in range(NT)])
        op0 = delayed(gen_outproj(0, False), 8)
        op1 = delayed(gen_outproj(1, True), 12)
        for _ in range(4):
            next(p0)  # proj(0) n=0
        interleave([(p0, 1), (a0, 2)], stop_idx=0)
        interleave([(a0, 3), (p1, 1), (op0, 1)], stop_idx=0)
        interleave([(a1, 3), (p1, 1), (op0, 1), (op1, 1)], stop_idx=0)
        run_all(op1)
    return nc


BF = ml_dtypes.bfloat16


def make_in_maps(x, Wq, Wk, Wv, Wo):
    # [B, S, E] -> [B, NT, E, 512] (token-block-tiled, feature-major)
    x_t = np.ascontiguousarray(
        np.transpose(
            np.asarray(x, np.float32).reshape(B, NT, 512, E), (0, 1, 3, 2)
        )
    ).astype(BF)
    Wq = np.asarray(Wq, np.float32)
    Wk = np.asarray(Wk, np.float32)
    Wv = np.asarray(Wv, np.float32)
    Wo = np.asarray(Wo, np.float32)
    # constants: [ident | tri | selc] bf16
    cst = np.zeros((P, 3 * P), np.float32)
    cst[:, 0:P] = np.eye(P)
    cst[:, P : 2 * P] = np.triu(np.ones((P, P)))  # tri[k, q] = 1 iff q >= k
    for r0 in (0, 64):
        cst[r0, 2 * P : 2 * P + 64] = 1.0
        cst[r0 + 32, 2 * P + 64 : 3 * P] = 1.0
    cst = cst.astype(BF)
    in_maps = []
    for c in range(NCORES):
        wq_sh = np.ascontiguousarray(Wq[:, FPC * c : FPC * (c + 1)]).astype(BF)
        wkv_sh = np.concatenate(
            [Wk[:, D * c : D * (c + 1)], Wv[:, D * c : D * (c + 1)]], axis=1
        ).astype(BF)
        wo_sh = np.ascontiguousarray(Wo[FPC * c : FPC * (c + 1), :]).astype(BF)
        in_maps.append(
            {"x_t": x_t, "wq": wq_sh, "wkv": wkv_sh, "wo": wo_sh, "cst": cst}
        )
    return in_maps


_NC_CACHE = {}


def get_nc():
    if "nc" not in _NC_CACHE:
        nc = build_nc()
        nc.compile()
        _NC_CACHE["nc"] = nc
    return _NC_CACHE["nc"]


def kernel(x, Wq, Wk, Wv, Wo, bo, mask=None, **_ignored):
    nc = get_nc()
    in_maps = make_in_maps(x, Wq, Wk, Wv, Wo)
    res = run_bass_kernel_spmd(nc, in_maps, list(range(NCORES)))
    total = np.zeros((B, KT, 2, P, 1024), np.float32)
    for c in range(NCORES):
        total += np.asarray(res.results[c]["out"], np.float32)
    # [B, KT, 2, 128, 1024] -> [B, S, E]: feature = m*128+p, token = np*1024+s
    full = np.transpose(total, (0, 2, 4, 1, 3)).reshape(B, S, E)
    full = full + np.asarray(bo, np.float32)[None, None, :]
    return np.ascontiguousarray(full)


# revision 33
# speedup vs baseline: 1.0174x; 1.0082x over previous
"""Distributed GQA attention block for Trainium2 (8 NeuronCores).

Problem: nn_Attention_65927747993826
  x:[2,2048,2048] f32, causal GQA attention, H=32 query heads, G=8 KV groups,
  head_size=64, with q/k/v/out projections and bias.

Sharding (8-way head parallel): core c owns query heads [4c, 4c+4) and KV
group c. Each core computes q/k/v projections for its heads from the full x,
causal flash-attention for its 4 heads, and a partial output projection
through its 256 rows of Wo. The host sums the 8 partial outputs and adds the
bias (a per-feature constant commutes with the partial-sum reduction).

Layouts on chip are feature-major ("transposed"): x^T [E, S] etc., so every
matmul contracts over the partition dim with zero on-chip transposes except
v (PE-transposed). Compute dtype bf16 (f32 accumulate in PSUM).

Head-PAIR packing: q is stored with head 2j at partitions 0-63 and head
2j+1 at 64-127 (k duplicated across both halves), so the two heads' score
matmuls are K=64 row-tiles at tile_position (0,0)/(64,0) that execute
CONCURRENTLY on the PE's 32x32 subarrays — halving score cost vs the old
zero-padded K=128 scheme. Softmax denominators ride the ones-column of
v_aug; normalization is a reciprocal_approx_fast + a K=2 PE "broadcast"
matmul against a selector matrix + one full-width DVE multiply per chunk.
"""

from contextlib import ExitStack

import numpy as np
import ml_dtypes

import concourse.bass as bass
import concourse.mybir as mybir
import concourse.tile as tile
from concourse import bacc
from concourse.bass import ts, ds
from concourse.bass_utils import run_bass_kernel_spmd

B, S, E = 2, 2048, 2048
H, G, D = 32, 8, 64
NCORES = 8
HPC = H // NCORES            # query heads per core: 4
FPC = HPC * D                # q features per core: 256
P = 128
KT = E // P                  # 16 contraction tiles over E
NT = S // 512                # 4 token 512-blocks per batch
SCALE = D ** -0.5
F32 = mybir.dt.float32
BF16 = mybir.dt.bfloat16
FA = mybir.ActivationFunctionType
ALU = mybir.AluOpType


def build_nc():
    nc = bacc.Bacc()
    # x_t tiled [B, NT, E, 512]: each (b, n) token-block is a dense 2MB
    # region so the strided per-partition DMA rows stay page-local
    x_t = nc.declare_dram_parameter("x_t", [B, NT, E, 512], BF16, isOutput=False)
    wq = nc.declare_dram_parameter("wq", [E, FPC], BF16, isOutput=False)
    wkv = nc.declare_dram_parameter("wkv", [E, P], BF16, isOutput=False)
    wo = nc.declare_dram_parameter("wo", [FPC, E], BF16, isOutput=False)
    # host-baked constants: identity (v transposes), upper-triangular causal
    # mask, and the selector matrix for the normalize broadcast matmuls
    cst = nc.declare_dram_parameter("cst", [P, 3 * P], BF16, isOutput=False)
    # out tiled [B, KT, 2, 128, 1024]: every output DMA is one contiguous
    # 256KB block (two token-blocks per store); the host reassembles
    out = nc.declare_dram_parameter("out", [B, KT, 2, P, 1024], BF16, isOutput=True)

    with ExitStack() as ctx:
        tc = ctx.enter_context(tile.TileContext(nc))
        consts = ctx.enter_context(tc.tile_pool(name="consts", bufs=1))
        wpool = ctx.enter_context(tc.tile_pool(name="w", bufs=1))
        xbp = ctx.enter_context(tc.tile_pool(name="xb", bufs=4))
        qkvp = ctx.enter_context(tc.tile_pool(name="qkv", bufs=2))
        ppool = ctx.enter_context(tc.tile_pool(name="probs", bufs=14))
        npool = ctx.enter_context(tc.tile_pool(name="norm", bufs=4))
        opool = ctx.enter_context(tc.tile_pool(name="outsb", bufs=4))
        pp_mm = ctx.enter_context(tc.tile_pool(name="pmm", bufs=2, space="PSUM"))
        pp_sp = ctx.enter_context(tc.tile_pool(name="psp", bufs=2, space="PSUM"))
        pp_acc = ctx.enter_context(tc.tile_pool(name="pacc", bufs=2, space="PSUM"))

        # ---- critical-path weight chunk first: the opening matmul only
        # needs wq k-tiles 0-1 ----
        wq_sb = wpool.tile([P, KT, FPC], BF16)
        nc.gpsimd.dma_start(
            wq_sb[:, 0:2, :], wq[0:256, :].rearrange("(ko p) m -> p ko m", p=P)
        )

        # ---- constants (host-baked, DMA'd on the gpsimd queue) ----
        cst_sb = consts.tile([P, 3 * P], BF16)
        nc.gpsimd.dma_start(cst_sb, cst[:, :])
        ident = cst_sb[:, 0:P]
        tri = cst_sb[:, P : 2 * P]  # tri[k, q] = 1 iff q >= k
        # selector for the softmax-normalize broadcast matmuls: for chunk
        # kk, bc[:, kk*512:] = selc[64kk:64kk+33].T @ rec[64kk:64kk+33] puts
        # rec row 64kk on partitions 0-63 and row 64kk+32 on 64-127 (den
        # rows sit at 32-aligned partitions; engine APs need 32-aligned
        # bases). selc's zero rows hit dq's memset-1.0 filler, never NaN.
        selc = cst_sb[:, 2 * P : 3 * P]

        # ---- remaining weights (gpsimd DMA queue, parallel to x on
        # sync/scalar/vector) ----
        nc.gpsimd.dma_start(
            wq_sb[:, 2:, :], wq[256:, :].rearrange("(ko p) m -> p ko m", p=P)
        )
        wkv_sb = wpool.tile([P, KT, P], BF16)
        nc.gpsimd.dma_start(wkv_sb, wkv.rearrange("(ko p) m -> p ko m", p=P))
        wo_sb = wpool.tile([P, 2, E], BF16)
        nc.gpsimd.dma_start(wo_sb, wo.rearrange("(ko p) m -> p ko m", p=P))

        state = {}
        xq = {}  # b -> prefetched half-tiles for the next token block

        def issue_x(b, n):
            """issue the two half-tile loads for block (b, n) on the sync
            hardware queue, a full block ahead of their consumer."""
            tiles = []
            for g in range(2):
                xb = xbp.tile([P, KT // 2, 512], BF16)
                nc.sync.dma_start(
                    xb,
                    x_t[b, n, ds(1024 * g, 1024), :].rearrange(
                        "(ko p) s -> p ko s", p=P
                    ),
                )
                tiles.append(xb)
            return tiles

        def gen_proj_n(b, n):
            """projection of token-block n for batch b (PE-heavy)."""
            if n == 0:
                # q4: head 2j at partitions 0-63, head 2j+1 at 64-127 so the
                # two heads' score matmuls row-tile the PE concurrently
                q4 = qkvp.tile([P, 2, S], BF16, tag="q4")
                # k2: k^T duplicated on both partition halves (scores lhsT)
                k2 = qkvp.tile([P, S], BF16, tag="k2")
                # kv: k^T rows 0-63, v^T rows 64-127 (v-transpose source)
                kvsb = qkvp.tile([P, S], BF16, tag="kv")
                # v token-major (+ ones column), PE-transposed per block
                vsb = qkvp.tile([P, S // P, D + 1], BF16, tag="v")
                nc.gpsimd.memset(vsb[:, :, D : D + 1], 1.0)
                # attnsb holds UNNORMALIZED attn until deferred normalize
                attnsb = qkvp.tile([P, 2, S], BF16, tag="attn")
                state[b] = (q4, k2, kvsb, vsb, attnsb)
            q4, k2, kvsb, vsb, attnsb = state[b]
            # x half-tiles are prefetched one block ahead (issued before
            # the PREVIOUS block's m-loop) so transfers hide under compute.
            # The very first block is chunked tile-by-tile across both
            # hardware DMA queues so the opening matmuls track arrivals.
            if b == 0 and n == 0:
                xh = []
                xb = xbp.tile([P, KT // 2, 512], BF16)
                for q_ in range(8):
                    (nc.sync if q_ % 2 == 0 else nc.scalar).dma_start(
                        xb[:, ds(q_, 1), :],
                        x_t[b, n, ds(q_ * 128, 128), :].rearrange(
                            "(ko p) s -> p ko s", p=P
                        ),
                    )
                xh.append(xb)
                xb = xbp.tile([P, KT // 2, 512], BF16)
                for q_ in range(4):
                    (nc.sync if q_ % 2 == 0 else nc.scalar).dma_start(
                        xb[:, ds(q_ * 2, 2), :],
                        x_t[b, n, ds(1024 + q_ * 256, 256), :].rearrange(
                            "(ko p) s -> p ko s", p=P
                        ),
                    )
                xh.append(xb)
            else:
                xh = xq[b]
            if n < 3:
                xq[b] = issue_x(b, n + 1)
            elif b == 0:
                xq[1] = issue_x(1, 0)
            if b == 0 and n == 0:
                # k-major so the PE consumes each arriving x chunk three
                # times over (q pair 0, q pair 1, kv) before waiting for the
                # next chunk; the two extra accumulators borrow the (idle)
                # score-PSUM pool
                ps0 = pp_mm.tile([P, 512], F32, tag="mm")
                ps12 = pp_sp.tile([P, 1024], F32, tag="sp")
                psl = [ps0, ps12[:, ts(0, 512)], ps12[:, ts(1, 512)]]
                for k in range(KT):
                    for m in range(3):
                        lhsT = wq_sb[:, k, ts(m, P)] if m < 2 else wkv_sb[:, k, :]
                        nc.tensor.matmul(
                            psl[m],
                            lhsT,
                            xh[k // 8][:, k % 8, :],
                            start=(k == 0),
                            stop=(k == KT - 1),
                        )
                    if k % 8 == 5:
                        yield
                for m in range(3):
                    ps = psl[m]
                    if m < 2:
                        nc.vector.tensor_copy(q4[:, m, ts(n, 512)], ps)
                    else:
                        nc.vector.tensor_copy(kvsb[:, ts(n, 512)], ps)
                        nc.vector.tensor_copy(k2[0:64, ts(n, 512)], ps[0:64, :])
                        nc.vector.tensor_copy(k2[64:128, ts(n, 512)], ps[0:64, :])
                yield
            else:
                for m in range(3):
                    ps = pp_mm.tile([P, 512], F32, tag="mm")
                    for k in range(KT):
                        lhsT = wq_sb[:, k, ts(m, P)] if m < 2 else wkv_sb[:, k, :]
                        nc.tensor.matmul(
                            ps,
                            lhsT,
                            xh[k // 8][:, k % 8, :],
                            start=(k == 0),
                            stop=(k == KT - 1),
                        )
                    if m < 2:
                        # heads 2m (rows 0-63) and 2m+1 (rows 64-127) in place
                        nc.vector.tensor_copy(q4[:, m, ts(n, 512)], ps)
                    else:
                        nc.vector.tensor_copy(kvsb[:, ts(n, 512)], ps)
                        nc.vector.tensor_copy(k2[0:64, ts(n, 512)], ps[0:64, :])
                        nc.vector.tensor_copy(k2[64:128, ts(n, 512)], ps[0:64, :])
                    yield
            # v transposes for this token-block, one group per n so the
            # attention for q-block n can start as soon as proj n is done
            for st in range(4 * n, 4 * n + 4):
                tp = pp_mm.tile([P, P], BF16, tag="mm")
                nc.tensor.transpose(tp, kvsb[:, ts(st, P)], ident)
                nc.vector.tensor_copy(vsb[:, st, 0:D], tp[:, 64:128])
            yield

        def gen_att_qt(b, qt):
            """causal attention for q-block qt of batch b (ACT-heavy: exp).

            Heads run in pairs: head 2j streams from partitions 0-63, head
            2j+1 from 64-127, two concurrent K=64 PE row-tiles per k-tile.
            softmax denominators come for free from the ones-column of v_aug
            (row 64 of each accumulator); normalization is deferred to a
            batched reciprocal + PE-broadcast per q-block.
            """
            q4, k2, kvsb, vsb, attnsb = state[b]
            dq = npool.tile([P, 512], F32, tag="den")
            nc.gpsimd.memset(dq, 1.0)
            nfull = 4 * qt
            for j in range(2):
                accA = pp_acc.tile([D + 1, 512], F32, tag="acc")
                accB = pp_acc.tile([D + 1, 512], F32, tag="acc")
                # 1) diagonal scores first: their exp+mask chains get
                #    maximum slack before their attnV consumers issue last.
                #    Head A lands at sp cols [0,w), head B at [512,512+w);
                #    one exp ACTIVATE spans both (the gap cols are garbage
                #    that nothing reads).
                diag_prs = []
                for t in range(4):
                    kt = nfull + t
                    off = t * P
                    w_ = 512 - off
                    sp = pp_sp.tile([P, 1024], F32, tag="sp")
                    nc.tensor.matmul(
                        sp[:, ds(0, w_)],
                        k2[0:64, ts(kt, P)],
                        q4[0:64, j, ds(512 * qt + off, w_)],
                        start=True,
                        stop=True,
                    )
                    nc.tensor.matmul(
                        sp[:, ds(512, w_)],
                        k2[64:128, ts(kt, P)],
                        q4[64:128, j, ds(512 * qt + off, w_)],
                        start=True,
                        stop=True,
                    )
                    pr = ppool.tile([P, 1024], BF16, tag="pr")
                    diag_prs.append(pr)
                    nc.scalar.activation(
                        pr[:, ds(0, 512 + w_)],
                        sp[:, ds(0, 512 + w_)],
                        FA.Exp,
                        scale=SCALE,
                    )
                    nc.vector.tensor_tensor(
                        pr[:, ds(0, P)], pr[:, ds(0, P)], tri, ALU.mult
                    )
                    nc.vector.tensor_tensor(
                        pr[:, ds(512, P)], pr[:, ds(512, P)], tri, ALU.mult
                    )
                yield
                # 2) full (off-diagonal) k-tiles: one [128,1024] PSUM tile
                #    holds both heads' scores for k-tile kt; one exp covers
                #    both; attnV for each head follows immediately
                firstA = firstB = True
                for kt in range(nfull):
                    sp = pp_sp.tile([P, 1024], F32, tag="sp")
                    nc.tensor.matmul(
                        sp[:, ts(0, 512)],
                        k2[0:64, ts(kt, P)],
                        q4[0:64, j, ts(qt, 512)],
                        start=True,
                        stop=True,
                    )
                    nc.tensor.matmul(
                        sp[:, ts(1, 512)],
                        k2[64:128, ts(kt, P)],
                        q4[64:128, j, ts(qt, 512)],
                        start=True,
                        stop=True,
                    )
                    pr = ppool.tile([P, 1024], BF16, tag="pr")
                    nc.scalar.activation(pr, sp, FA.Exp, scale=SCALE)
                    nc.tensor.matmul(
                        accA, vsb[:, kt, :], pr[:, ts(0, 512)],
                        start=firstA, stop=False,
                    )
                    firstA = False
                    nc.tensor.matmul(
                        accB, vsb[:, kt, :], pr[:, ts(1, 512)],
                        start=firstB, stop=False,
                    )
                    firstB = False
                    if kt % 2 == 1:
                        yield
                # 3) diagonal attnV last
                for t in range(4):
                    kt = nfull + t
                    off = t * P
                    w_ = 512 - off
                    nc.tensor.matmul(
                        accA[:, ds(off, w_)],
                        vsb[:, kt, :],
                        diag_prs[t][:, ds(0, w_)],
                        start=firstA,
                        stop=(t == 3),
                    )
                    firstA = False
                    nc.tensor.matmul(
                        accB[:, ds(off, w_)],
                        vsb[:, kt, :],
                        diag_prs[t][:, ds(512, w_)],
                        start=firstB,
                        stop=(t == 3),
                    )
                    firstB = False
                yield
                # evacuate unnormalized attn + denominator rows (chunk j's
                # denominators land at 32-aligned partitions 64j/64j+32 for
                # the selector-matmul broadcast below)
                nc.vector.tensor_copy(attnsb[0:64, j, ts(qt, 512)], accA[0:D, :])
                nc.vector.tensor_copy(dq[64 * j : 64 * j + 1, :], accA[D : D + 1, :])
                nc.vector.tensor_copy(
                    attnsb[64:128, j, ts(qt, 512)], accB[0:D, :]
                )
                nc.vector.tensor_copy(
                    dq[64 * j + 32 : 64 * j + 33, :], accB[D : D + 1, :]
                )
                yield
            # deferred softmax normalization for this q-block: fast approx
            # reciprocal (denominators are >= e^-8, ~51 ULP is plenty), then
            # two concurrent K=2 broadcast matmuls spread the four rec rows
            # across the partitions of both chunks, then two full-width mults
            rec = npool.tile([P, 512], F32, tag="rec")
            nc.vector.reciprocal_approx_fast(out=rec, in_=dq)
            # bf16 copy for the broadcast matmuls (fp32 PE matmul is 2-pass)
            recb = npool.tile([P, 512], BF16, tag="recb")
            nc.vector.tensor_copy(recb, rec)
            bc = pp_sp.tile([P, 1024], F32, tag="sp")
            nc.tensor.matmul(
                bc[:, ts(0, 512)], selc[0:33, :], recb[0:33, :],
                start=True, stop=True,
            )
            nc.tensor.matmul(
                bc[:, ts(1, 512)], selc[64:97, :], recb[64:97, :],
                start=True, stop=True,
            )
            for kk in range(2):
                dst = attnsb[:, kk, ts(qt, 512)]
                nc.vector.tensor_tensor(dst, dst, bc[:, ts(kk, 512)], ALU.mult)
            yield

        def gen_outproj(b, use_act):
            """partial output projection for batch b (PE-heavy).

            npair-outer so the later token blocks' softmax-normalize latency
            is hidden behind the first pair's matmuls. use_act alternates the
            PSUM evacuation onto ScalarE only when no attention phase is
            keeping ScalarE saturated with exps. Each store covers two token
            blocks (256KB) to halve the sync queue's DMA-issue load.
            """
            attnsb = state[b][4]
            for np_ in range(2):
                for m in range(KT):
                    osb = opool.tile([P, 1024], BF16)
                    for nn in range(2):
                        po = pp_mm.tile([P, 512], F32, tag="mm")
                        for kk in range(2):
                            nc.tensor.matmul(
                                po,
                                wo_sb[:, kk, ts(m, P)],
                                attnsb[:, kk, ts(2 * np_ + nn, 512)],
                                start=(kk == 0),
                                stop=(kk == 1),
                            )
                        if (m + nn) % 3 == 2 or (use_act and (m + nn) % 3 == 1):
                            nc.scalar.copy(osb[:, ts(nn, 512)], po)
                        else:
                            nc.vector.tensor_copy(osb[:, ts(nn, 512)], po)
                    nc.sync.dma_start(out[b, m, np_, :, :], osb)
                    if m % 2 == 1:
                        yield

        def run_all(gen):
            for _ in gen:
                pass

        def interleave(pairs, stop_idx=None):
            """pairs: list of [gen, steps_per_round]. Round-robin with ratios
            so the PE-filler generator is spread across the whole phase.
            With stop_idx set, the phase ends when that generator exhausts
            (the others keep their state for the next phase)."""
            live = [[g, r] for g, r in pairs]
            stop = live[stop_idx][0] if stop_idx is not None else None
            while live:
                for gr in live[:]:
                    try:
                        for _ in range(gr[1]):
                            next(gr[0])
                    except StopIteration:
                        live.remove(gr)
                        if gr[0] is stop:
                            return

        def delayed(gen, k):
            for _ in range(k):
                yield
            yield from gen

        def chain(gens):
            for g in gens:
                yield from g

        # Pipeline projections, attention, and out-projections so PE-heavy
        # matmul work fills the PE bubbles of the ACT(exp)-bound attention
        # phases. Attention for a batch trails its projection by ~1 token
        # block; out-projections enter delayed so their matmuls trail the
        # q-block normalizes they depend on in the in-order PE stream
        # (too-small delays DEADLOCK: PE waits on a DVE normalize whose
        # feeding matmul sits behind it in the queue).
        # NB: abandoned delayed() wrappers must keep being stepped by name —
        # letting one get garbage-collected closes the delegated generator.
        p0 = chain([gen_proj_n(0, n) for n in range(NT)])
        p1 = chain([gen_proj_n(1, n) for n in range(NT)])
        a0 = delayed(chain([gen_att_qt(0, qt) for qt in range(NT)]), 6)
        a1 = chain([gen_att_qt(1, qt) for qt in range(NT)])
        op0 = delayed(gen_outproj(0, False), 8)
        op1 = delayed(gen_outproj(1, True), 12)
        for _ in range(4):
            next(p0)  # proj(0) n=0
        interleave([(p0, 1), (a0, 2)], stop_idx=0)
        interleave([(a0, 3), (p1, 1), (op0, 1)], stop_idx=0)
        interleave([(a1, 3), (p1, 1), (op0, 1), (op1, 1)], stop_idx=0)
        run_all(op1)
    return nc


BF = ml_dtypes.bfloat16


def make_in_maps(x, Wq, Wk, Wv, Wo):
    # [B, S, E] -> [B, NT, E, 512] (token-block-tiled, feature-major)
    x_t = np.ascontiguousarray(
        np.transpose(
            np.asarray(x, np.float32).reshape(B, NT, 512, E), (0, 1, 3, 2)
        )
    ).astype(BF)
    Wq = np.asarray(Wq, np.float32)
    Wk = np.asarray(Wk, np.float32)
    Wv = np.asarray(Wv, np.float32)
    Wo = np.asarray(Wo, np.float32)
    # constants: [ident | tri | selc] bf16
    cst = np.zeros((P, 3 * P), np.float32)
    cst[:, 0:P] = np.eye(P)
    cst[:, P : 2 * P] = np.triu(np.ones((P, P)))  # tri[k, q] = 1 iff q >= k
    for r0 in (0, 64):
        cst[r0, 2 * P : 2 * P + 64] = 1.0
        cst[r0 + 32, 2 * P + 64 : 3 * P] = 1.0
    cst = cst.astype(BF)
    in_maps = []
    for c in range(NCORES):
        wq_sh = np.ascontiguousarray(Wq[:, FPC * c : FPC * (c + 1)]).astype(BF)
        wkv_sh = np.concatenate(
            [Wk[:, D * c : D * (c + 1)], Wv[:, D * c : D * (c + 1)]], axis=1
        ).astype(BF)
        wo_sh = np.ascontiguousarray(Wo[FPC * c : FPC * (c + 1), :]).astype(BF)
        in_maps.append(
            {"x_t": x_t, "wq": wq_sh, "wkv": wkv_sh, "wo": wo_sh, "cst": cst}
        )
    return in_maps


_NC_CACHE = {}


def get_nc():
    if "nc" not in _NC_CACHE:
        nc = build_nc()
        nc.compile()
        _NC_CACHE["nc"] = nc
    return _NC_CACHE["nc"]


def kernel(x, Wq, Wk, Wv, Wo, bo, mask=None, **_ignored):
    nc = get_nc()
    in_maps = make_in_maps(x, Wq, Wk, Wv, Wo)
    res = run_bass_kernel_spmd(nc, in_maps, list(range(NCORES)))
    total = np.zeros((B, KT, 2, P, 1024), np.float32)
    for c in range(NCORES):
        total += np.asarray(res.results[c]["out"], np.float32)
    # [B, KT, 2, 128, 1024] -> [B, S, E]: feature = m*128+p, token = np*1024+s
    full = np.transpose(total, (0, 2, 4, 1, 3)).reshape(B, S, E)
    full = full + np.asarray(bo, np.float32)[None, None, :]
    return np.ascontiguousarray(full)
